# revision 27
# baseline (speedup 1.0000x reference)
"""DirectionalContrastiveLoss on 8 TRN2 NeuronCores (Bass/Tile).

Key optimization over the dense data-parallel version: the loss only
involves anchor rows whose pos-mask is nonzero, and the masks depend
only on the host-visible pseudo_logits:
    pos_mask_1 = (g2 > 0.7) & (g1 < g2)   ~26% of rows
    pos_mask_2 = (g1 > 0.7) & (g2 < g1)   ~26% of rows (disjoint from 1)
So the host compacts the ~52% union of masked rows into one unified
anchor stream (branch-1 rows use feat1 as anchor/label1, branch-2 rows
use feat2/label2; pos = <f1,f2>/TEMP is shared and precomputed on the
host in fp64).  This cuts PE matmul, ScalarE exp, and DVE reduce work
~3.6x vs processing all 16384 rows for both branches.

Device algorithm (validated against the fp64 reference in numcheck):
- sim = anchor @ memT/TEMP - 4000*eq computed on the PE in bf16; the
  label mask rides along as one accumulating matmul per 512-col chunk
  using the full 128-partition one-hot tensors (4 stacked copies at
  32-row offsets -> bias -4000, and a 128-row stationary keeps FWL on
  so LoadStationary never bubbles the PE stream).  exp(sim-4000) == 0
  in fp32, reproducing the reference's masked exp-sum.  Memory pad
  columns (4000->4096) carry onehot=1 in every label row so they
  vanish for every anchor.
- The softmax shift is pos (not the row max): rows where exp(sim-pos)
  overflows to +inf are provably dead (sim >= pos + 88 implies the
  true logit < e^-88, so -log(sigma+EPS) = -log(EPS) either way), and
  rows that matter can never overflow.
- The device returns the raw per-row masked exp-sums SS [128, NT]; the
  host computes -log(1/(SS+1+EPS)+EPS) and the two masked means in
  fp64 (cheap O(N), and it avoids an ACT table switch for Ln plus the
  whole on-device epilogue).

TRN2 clock (HAM) handling: the PE boost (1.2 -> 2.4 GHz) only engages
after sustained busy windows and drops permanently on sub-microsecond
PE gaps, so the kernel front-loads no-dependency garbage matmuls while
the DMAs stream in (plus per-unit insurance matmuls in the first two
tiles), and the steady-state pipeline is kept strictly PE-bound.
"""
from contextlib import ExitStack

import numpy as np
import ml_dtypes

TEMP = 0.1
POS_THRESH = 0.7
EPS = 1e-8
N, C, M, NLAB = 16384, 256, 4000, 21
MP = 4096                  # memory columns padded
NCORES = 8
NU = 4                     # psum units per n-tile
UNIT = MP // NU            # 1024 (= 2 PSUM banks, J=512 chunks)

_cache = {}


def _build(NT):
    import concourse.bacc as bacc
    import concourse.tile as tile
    from concourse import mybir

    f32 = mybir.dt.float32
    bf16 = mybir.dt.bfloat16
    Act = mybir.ActivationFunctionType
    X = mybir.AxisListType.X

    RPC = NT * 128             # compacted rows per core

    # Bacc (not raw Bass): its finalize() runs generate_event_semaphores(),
    # which splits multi-sem waits into EVSEM chains — walrus allows at most
    # one sync-wait per instruction.
    nc = bacc.Bacc(None)

    exta_d = nc.declare_dram_parameter("exta", [C, RPC], bf16, isOutput=False)
    eqa_d = nc.declare_dram_parameter("eqanc", [128, RPC], bf16, isOutput=False)
    mem_d = nc.declare_dram_parameter("extmem", [C, MP], bf16, isOutput=False)
    eqm_d = nc.declare_dram_parameter("eqmem", [128, MP], bf16, isOutput=False)
    npos_d = nc.declare_dram_parameter("npos", [128, NT], f32, isOutput=False)
    out_d = nc.declare_dram_parameter("out", [128, NT], f32, isOutput=True)

    with tile.TileContext(nc) as tc, ExitStack() as ctx:
        consts = ctx.enter_context(tc.tile_pool(name="consts", bufs=1))
        small = ctx.enter_context(tc.tile_pool(name="small", bufs=3))
        psum = ctx.enter_context(
            tc.tile_pool(name="psum", bufs=NU, space="PSUM")
        )

        # ---- resident inputs ----
        # Order matters: NPOS (gates the first ScalarE exp) and tile-0's
        # matmul operands load first in small dedicated tiles, then the
        # bulk tensors.
        NPOS = consts.tile([128, NT], f32, tag="NPOS", name="NPOS")
        nc.sync.dma_start(out=NPOS[:], in_=npos_d[:])

        ea0_k, eqa0 = [], None
        for i in range(2):
            k0, k1 = i * 128, (i + 1) * 128
            t0t = consts.tile([128, 128], bf16, tag=f"ea0_{i}", name=f"ea0_{i}")
            nc.sync.dma_start(out=t0t[:], in_=exta_d[k0:k1, 0:128])
            ea0_k.append(t0t)
        eqa0 = consts.tile([128, 128], bf16, tag="eqa0", name="eqa0")
        nc.sync.dma_start(out=eqa0[:], in_=eqa_d[:, 0:128])

        memc = [[None] * NU for _ in range(2)]
        eqmc = [None] * NU
        ea_k = []
        for u in range(NU):
            c0, c1 = u * UNIT, (u + 1) * UNIT
            for i in range(2):
                k0, k1 = i * 128, (i + 1) * 128
                mt = consts.tile([128, UNIT], bf16, tag=f"mem{i}u{u}",
                                 name=f"mem{i}u{u}")
                nc.sync.dma_start(out=mt[:], in_=mem_d[k0:k1, c0:c1])
                memc[i][u] = mt
            et = consts.tile([128, UNIT], bf16, tag=f"eqmu{u}", name=f"eqmu{u}")
            nc.sync.dma_start(out=et[:], in_=eqm_d[:, c0:c1])
            eqmc[u] = et
            if u == 0:
                # the full anchor tensors feed every tile from t=1 on; get
                # them in flight right after tile-0's own dependencies.
                for i in range(2):
                    k0, k1 = i * 128, (i + 1) * 128
                    t1 = consts.tile([128, RPC], bf16, tag=f"ea_{i}",
                                     name=f"ea_{i}")
                    nc.sync.dma_start(out=t1[:], in_=exta_d[k0:k1, :])
                    ea_k.append(t1)
                eqa = consts.tile([128, RPC], bf16, tag="eqa", name="eqa")
                nc.sync.dma_start(out=eqa[:], in_=eqa_d[:])

        # ---- HAM warm-up ballast ----
        # The PE only un-throttles from 1.2 to 2.4 GHz when it sees
        # sustained busy windows, and sub-us gaps re-throttle it (often
        # permanently for a light kernel).  Garbage matmuls with no data
        # deps keep the PE lit while the real inputs stream in.
        dmm = consts.tile([128, 1024], bf16, tag="dmm", name="dmm")
        nc.vector.memset(dmm[:], 0.0)
        dvedum = consts.tile([128, 2048], f32, tag="dvedum", name="dvedum")
        nc.vector.memset(dvedum[:], 1.0)
        dscr = consts.tile([128, 1], f32, tag="dscr", name="dscr")
        pdum = psum.tile([128, UNIT], f32, tag="pu", name="pdum")
        for i in range(16):
            j = i % 2
            nc.tensor.matmul(
                pdum[:, j * 512: (j + 1) * 512],
                dmm[:, 0:128],
                dmm[:, 0:512],
                start=True,
                stop=True,
            )
        for i in range(6):
            nc.vector.reduce_sum(out=dscr[:, 0:1], in_=dvedum[:], axis=X)

        SS = consts.tile([128, NT], f32, tag="SS", name="SS")
        for t in range(NT):
            tc0, tc1 = t * 128, (t + 1) * 128
            pu = [
                psum.tile([128, UNIT], f32, tag="pu", name=f"pu{t}_{u}")
                for u in range(NU)
            ]
            S = small.tile([128, NU], f32, tag="S", name=f"S{t}")
            ea_u = ea0_k if t == 0 else ea_k
            eqa_u = eqa0 if t == 0 else eqa
            ec0, ec1 = (0, 128) if t == 0 else (tc0, tc1)
            # Per-unit: dense K=256 bf16 (2 k-tiles), the -4000*eq one-hot
            # mask matmul (full 128 rows -> FWL stays on), then exp on
            # ScalarE (in place) and row-sum on VectorE while the PE moves
            # on to the next unit.
            for u in range(NU):
                if t < 3:
                    # handoff insurance: a no-dep garbage matmul keeps the
                    # PE stream gapless if this unit's inputs are still in
                    # flight (the real matmul below overwrites the region).
                    nc.tensor.matmul(
                        pu[u][:, 0:512],
                        dmm[:, 0:128],
                        dmm[:, 0:512],
                        start=True,
                        stop=True,
                    )
                for kt in range(2):
                    lhsT = ea_u[kt][:, ec0:ec1]
                    for j in range(2):
                        nc.tensor.matmul(
                            pu[u][:, j * 512: (j + 1) * 512],
                            lhsT,
                            memc[kt][u][:, j * 512: (j + 1) * 512],
                            start=(kt == 0),
                            stop=False,
                        )
                for j in range(2):
                    nc.tensor.matmul(
                        pu[u][:, j * 512: (j + 1) * 512],
                        eqa_u[:, ec0:ec1],
                        eqmc[u][:, j * 512: (j + 1) * 512],
                        start=False,
                        stop=True,
                    )
                nc.scalar.activation(
                    out=pu[u][:],
                    in_=pu[u][:],
                    func=Act.Exp,
                    bias=NPOS[:, t: t + 1],
                    scale=1.0,
                )
            for u in range(NU):
                nc.vector.reduce_sum(out=S[:, u: u + 1], in_=pu[u][:], axis=X)
            nc.vector.reduce_sum(out=SS[:, t: t + 1], in_=S[:], axis=X)

        nc.sync.dma_start(out=out_d[:], in_=SS[:])

    nc.finalize()
    return nc


def _host_prep(inputs):
    bf = ml_dtypes.bfloat16
    f1 = np.ascontiguousarray(np.asarray(inputs["output_feat1"], np.float32))
    f2 = np.ascontiguousarray(np.asarray(inputs["output_feat2"], np.float32))
    l1 = np.asarray(inputs["pseudo_label1"], np.int64)
    l2 = np.asarray(inputs["pseudo_label2"], np.int64)
    g1 = np.asarray(inputs["pseudo_logits1"], np.float32)
    g2 = np.asarray(inputs["pseudo_logits2"], np.float32)
    ul1 = np.asarray(inputs["output_ul1"], np.float32)
    ul2 = np.asarray(inputs["output_ul2"], np.float32)
    i1 = np.asarray(inputs["selected_idx1"], np.int64)
    i2 = np.asarray(inputs["selected_idx2"], np.int64)

    b, c, h, w = ul1.shape
    u1 = ul1.transpose(0, 2, 3, 1).reshape(b * h * w, c)
    u2 = ul2.transpose(0, 2, 3, 1).reshape(b * h * w, c)
    mem = np.concatenate([u1[i1], u2[i2]], axis=0)               # [M, C]
    memlab = np.concatenate([l1[i1], l2[i2]], axis=0)            # [M]

    # --- compact the masked rows of both branches into one stream ---
    m1 = (g2 > POS_THRESH) & (g1 < g2)
    m2 = (g1 > POS_THRESH) & (g2 < g1)
    idx1 = np.nonzero(m1)[0]
    idx2 = np.nonzero(m2)[0]
    n1, n2 = len(idx1), len(idx2)
    R = n1 + n2
    NT = max(1, (((R + 127) // 128) + NCORES - 1) // NCORES)
    RT = NT * 128 * NCORES

    pos_full = (
        np.sum(f1.astype(np.float64) * f2.astype(np.float64), axis=1) / TEMP
    ).astype(np.float32)

    anchors = np.zeros((RT, C), np.float32)
    alab = np.zeros(RT, np.int64)
    posv = np.zeros(RT, np.float32)
    anchors[:n1] = f1[idx1]
    alab[:n1] = l1[idx1]
    posv[:n1] = pos_full[idx1]
    anchors[n1:R] = f2[idx2]
    alab[n1:R] = l2[idx2]
    posv[n1:R] = pos_full[idx2]

    lab_eye = np.arange(NLAB, dtype=np.int64)

    extmem = np.zeros((C, MP), np.float32)
    extmem[:, :M] = mem.T / TEMP
    extmem = extmem.astype(bf)                                   # [256, MP]

    oh_mem = np.zeros((NLAB, MP), np.float32)
    oh_mem[:, :M] = (memlab[None, :] == lab_eye[:, None])
    oh_mem[:, M:] = 1.0          # pad columns masked for every label
    eqmem = np.zeros((128, MP), np.float32)
    for i in range(NU):
        eqmem[32 * i: 32 * i + NLAB] = oh_mem
    eqmem = eqmem.astype(bf)                                     # [128, MP]

    oh_anc = -1000.0 * (alab[None, :] == lab_eye[:, None])       # [21, RT]
    eqa_full = np.zeros((128, RT), np.float32)
    for i in range(NU):
        eqa_full[32 * i: 32 * i + NLAB] = oh_anc
    eqa_full = eqa_full.astype(bf)

    exta = np.ascontiguousarray(anchors.T).astype(bf)            # [256, RT]

    RPC = NT * 128

    def pack_vec(v):    # [RPC] -> [128, NT]
        return np.ascontiguousarray(v.reshape(NT, 128).T)

    in_maps = []
    for cix in range(NCORES):
        sl = slice(cix * RPC, (cix + 1) * RPC)
        in_maps.append({
            "exta": np.ascontiguousarray(exta[:, sl]),
            "eqanc": np.ascontiguousarray(eqa_full[:, sl]),
            "extmem": extmem,
            "eqmem": eqmem,
            "npos": pack_vec(-posv[sl]),
        })
    return in_maps, NT, n1, n2


def _finalize(results, NT, n1, n2):
    RPC = NT * 128
    num1 = num2 = 0.0
    for cix, r in enumerate(results):
        ss = np.asarray(r["out"], np.float64)       # [128, NT]
        v = ss.T.reshape(RPC)                       # unpack pack_vec
        gidx = np.arange(cix * RPC, (cix + 1) * RPC)
        with np.errstate(divide="ignore", over="ignore"):
            lam = -np.log(1.0 / (v + 1.0 + EPS) + EPS)
        num1 += lam[gidx < n1].sum()
        num2 += lam[(gidx >= n1) & (gidx < n1 + n2)].sum()
    loss = num1 / (n1 + 1e-12) + num2 / (n2 + 1e-12)
    return np.float32(loss)


def _run(inputs, trace=False):
    from concourse.bass_utils import run_bass_kernel_spmd

    in_maps, NT, n1, n2 = _host_prep(inputs)
    if NT not in _cache:
        _cache[NT] = _build(NT)
    res = run_bass_kernel_spmd(
        _cache[NT], in_maps, list(range(NCORES)), trace=trace
    )
    return _finalize(res.results, NT, n1, n2), res


def kernel(**inputs):
    out, _ = _run(inputs)
    return out


def kernel_with_profile(**inputs):
    out, res = _run(inputs, trace=True)
    return out, res


# revision 29
# speedup vs baseline: 1.0397x; 1.0397x over previous
"""DirectionalContrastiveLoss on 8 TRN2 NeuronCores (Bass/Tile).

Key optimization over the dense data-parallel version: the loss only
involves anchor rows whose pos-mask is nonzero, and the masks depend
only on the host-visible pseudo_logits:
    pos_mask_1 = (g2 > 0.7) & (g1 < g2)   ~26% of rows
    pos_mask_2 = (g1 > 0.7) & (g2 < g1)   ~26% of rows (disjoint from 1)
So the host compacts the ~52% union of masked rows into one unified
anchor stream (branch-1 rows use feat1 as anchor/label1, branch-2 rows
use feat2/label2; pos = <f1,f2>/TEMP is shared and precomputed on the
host in fp64).  This cuts PE matmul, ScalarE exp, and DVE reduce work
~3.6x vs processing all 16384 rows for both branches.

Device algorithm (validated against the fp64 reference in numcheck):
- sim = anchor @ memT/TEMP - 4000*eq computed on the PE in bf16; the
  label mask rides along as one accumulating matmul per 512-col chunk
  using the full 128-partition one-hot tensors (4 stacked copies at
  32-row offsets -> bias -4000, and a 128-row stationary keeps FWL on
  so LoadStationary never bubbles the PE stream).  exp(sim-4000) == 0
  in fp32, reproducing the reference's masked exp-sum.  Memory pad
  columns (4000->4096) carry onehot=1 in every label row so they
  vanish for every anchor.
- The softmax shift is pos (not the row max): rows where exp(sim-pos)
  overflows to +inf are provably dead (sim >= pos + 88 implies the
  true logit < e^-88, so -log(sigma+EPS) = -log(EPS) either way), and
  rows that matter can never overflow.
- The device returns the raw per-row masked exp-sums SS [128, NT]; the
  host computes -log(1/(SS+1+EPS)+EPS) and the two masked means in
  fp64 (cheap O(N), and it avoids an ACT table switch for Ln plus the
  whole on-device epilogue).

TRN2 clock (HAM) handling: the PE boost (1.2 -> 2.4 GHz) only engages
after sustained busy windows and drops permanently on sub-microsecond
PE gaps, so the kernel front-loads no-dependency garbage matmuls while
the DMAs stream in (plus per-unit insurance matmuls in the first two
tiles), and the steady-state pipeline is kept strictly PE-bound.
"""
from contextlib import ExitStack

import numpy as np
import ml_dtypes

TEMP = 0.1
POS_THRESH = 0.7
EPS = 1e-8
N, C, M, NLAB = 16384, 256, 4000, 21
MP = 4096                  # memory columns padded
NCORES = 8
NU = 4                     # psum units per n-tile
UNIT = MP // NU            # 1024 (= 2 PSUM banks, J=512 chunks)

_cache = {}


def _build(NT):
    import concourse.bacc as bacc
    import concourse.tile as tile
    from concourse import mybir

    f32 = mybir.dt.float32
    bf16 = mybir.dt.bfloat16
    Act = mybir.ActivationFunctionType
    X = mybir.AxisListType.X

    RPC = NT * 128             # compacted rows per core

    # Bacc (not raw Bass): its finalize() runs generate_event_semaphores(),
    # which splits multi-sem waits into EVSEM chains — walrus allows at most
    # one sync-wait per instruction.
    nc = bacc.Bacc(None)

    exta_d = nc.declare_dram_parameter("exta", [C, RPC], bf16, isOutput=False)
    eqa_d = nc.declare_dram_parameter("eqanc", [128, RPC], bf16, isOutput=False)
    mem_d = nc.declare_dram_parameter("extmem", [C, MP], bf16, isOutput=False)
    eqm_d = nc.declare_dram_parameter("eqmem", [128, MP], bf16, isOutput=False)
    npos_d = nc.declare_dram_parameter("npos", [128, NT], f32, isOutput=False)
    out_d = nc.declare_dram_parameter("out", [128, NT], f32, isOutput=True)

    with tile.TileContext(nc) as tc, ExitStack() as ctx:
        consts = ctx.enter_context(tc.tile_pool(name="consts", bufs=1))
        small = ctx.enter_context(tc.tile_pool(name="small", bufs=3))
        psum = ctx.enter_context(
            tc.tile_pool(name="psum", bufs=NU, space="PSUM")
        )

        # ---- resident inputs ----
        # Order matters: NPOS (gates the first ScalarE exp) and tile-0's
        # matmul operands load first in small dedicated tiles, then the
        # bulk tensors.
        NPOS = consts.tile([128, NT], f32, tag="NPOS", name="NPOS")
        nc.sync.dma_start(out=NPOS[:], in_=npos_d[:])

        ea0_k, eqa0 = [], None
        for i in range(2):
            k0, k1 = i * 128, (i + 1) * 128
            t0t = consts.tile([128, 128], bf16, tag=f"ea0_{i}", name=f"ea0_{i}")
            nc.sync.dma_start(out=t0t[:], in_=exta_d[k0:k1, 0:128])
            ea0_k.append(t0t)
        eqa0 = consts.tile([128, 128], bf16, tag="eqa0", name="eqa0")
        nc.sync.dma_start(out=eqa0[:], in_=eqa_d[:, 0:128])

        memc = [[None] * NU for _ in range(2)]
        eqmc = [None] * NU
        for u in range(NU):
            c0, c1 = u * UNIT, (u + 1) * UNIT
            for i in range(2):
                k0, k1 = i * 128, (i + 1) * 128
                mt = consts.tile([128, UNIT], bf16, tag=f"mem{i}u{u}",
                                 name=f"mem{i}u{u}")
                nc.sync.dma_start(out=mt[:], in_=mem_d[k0:k1, c0:c1])
                memc[i][u] = mt
            et = consts.tile([128, UNIT], bf16, tag=f"eqmu{u}", name=f"eqmu{u}")
            nc.sync.dma_start(out=et[:], in_=eqm_d[:, c0:c1])
            eqmc[u] = et

        ea_k = []
        for i in range(2):
            k0, k1 = i * 128, (i + 1) * 128
            t1 = consts.tile([128, RPC], bf16, tag=f"ea_{i}", name=f"ea_{i}")
            nc.sync.dma_start(out=t1[:], in_=exta_d[k0:k1, :])
            ea_k.append(t1)
        eqa = consts.tile([128, RPC], bf16, tag="eqa", name="eqa")
        nc.sync.dma_start(out=eqa[:], in_=eqa_d[:])

        # ---- HAM warm-up ballast ----
        # The PE only un-throttles from 1.2 to 2.4 GHz when it sees
        # sustained busy windows, and sub-us gaps re-throttle it (often
        # permanently for a light kernel).  Garbage matmuls with no data
        # deps keep the PE lit while the real inputs stream in.
        # The first dummies read dmm UNINITIALIZED on purpose: waiting for a
        # DVE memset would delay the PE to the ~7us DVE bring-up, and the
        # garbage results land in PSUM regions that are never read before
        # being overwritten by a start=True matmul.
        dmm = consts.tile([128, 1024], bf16, tag="dmm", name="dmm")
        dvedum = consts.tile([128, 2048], f32, tag="dvedum", name="dvedum")
        dscr = consts.tile([128, 1], f32, tag="dscr", name="dscr")
        pdum = psum.tile([128, UNIT], f32, tag="pu", name="pdum")
        for i in range(16):
            j = i % 2
            nc.tensor.matmul(
                pdum[:, j * 512: (j + 1) * 512],
                dmm[:, 0:128],
                dmm[:, 0:512],
                start=True,
                stop=True,
            )
        nc.vector.memset(dmm[:], 0.0)
        nc.vector.memset(dvedum[:], 1.0)
        for i in range(6):
            nc.vector.reduce_sum(out=dscr[:, 0:1], in_=dvedum[:], axis=X)

        SS = consts.tile([128, NT], f32, tag="SS", name="SS")
        for t in range(NT):
            tc0, tc1 = t * 128, (t + 1) * 128
            pu = [
                psum.tile([128, UNIT], f32, tag="pu", name=f"pu{t}_{u}")
                for u in range(NU)
            ]
            S = small.tile([128, NU], f32, tag="S", name=f"S{t}")
            ea_u = ea0_k if t == 0 else ea_k
            eqa_u = eqa0 if t == 0 else eqa
            ec0, ec1 = (0, 128) if t == 0 else (tc0, tc1)
            # Per-unit: dense K=256 bf16 (2 k-tiles), the -4000*eq one-hot
            # mask matmul (full 128 rows -> FWL stays on), then exp on
            # ScalarE (in place) and row-sum on VectorE while the PE moves
            # on to the next unit.
            for u in range(NU):
                if t < 2:
                    # handoff insurance: a no-dep garbage matmul keeps the
                    # PE stream gapless if this unit's inputs are still in
                    # flight (the real matmul below overwrites the region).
                    nc.tensor.matmul(
                        pu[u][:, 0:512],
                        dmm[:, 0:128],
                        dmm[:, 0:512],
                        start=True,
                        stop=True,
                    )
                for kt in range(2):
                    lhsT = ea_u[kt][:, ec0:ec1]
                    for j in range(2):
                        nc.tensor.matmul(
                            pu[u][:, j * 512: (j + 1) * 512],
                            lhsT,
                            memc[kt][u][:, j * 512: (j + 1) * 512],
                            start=(kt == 0),
                            stop=False,
                        )
                for j in range(2):
                    nc.tensor.matmul(
                        pu[u][:, j * 512: (j + 1) * 512],
                        eqa_u[:, ec0:ec1],
                        eqmc[u][:, j * 512: (j + 1) * 512],
                        start=False,
                        stop=True,
                    )
                nc.scalar.activation(
                    out=pu[u][:],
                    in_=pu[u][:],
                    func=Act.Exp,
                    bias=NPOS[:, t: t + 1],
                    scale=1.0,
                )
            for u in range(NU):
                nc.vector.reduce_sum(out=S[:, u: u + 1], in_=pu[u][:], axis=X)
            nc.vector.reduce_sum(out=SS[:, t: t + 1], in_=S[:], axis=X)

        nc.sync.dma_start(out=out_d[:], in_=SS[:])

    nc.finalize()
    return nc


def _host_prep(inputs):
    bf = ml_dtypes.bfloat16
    f1 = np.ascontiguousarray(np.asarray(inputs["output_feat1"], np.float32))
    f2 = np.ascontiguousarray(np.asarray(inputs["output_feat2"], np.float32))
    l1 = np.asarray(inputs["pseudo_label1"], np.int64)
    l2 = np.asarray(inputs["pseudo_label2"], np.int64)
    g1 = np.asarray(inputs["pseudo_logits1"], np.float32)
    g2 = np.asarray(inputs["pseudo_logits2"], np.float32)
    ul1 = np.asarray(inputs["output_ul1"], np.float32)
    ul2 = np.asarray(inputs["output_ul2"], np.float32)
    i1 = np.asarray(inputs["selected_idx1"], np.int64)
    i2 = np.asarray(inputs["selected_idx2"], np.int64)

    b, c, h, w = ul1.shape
    u1 = ul1.transpose(0, 2, 3, 1).reshape(b * h * w, c)
    u2 = ul2.transpose(0, 2, 3, 1).reshape(b * h * w, c)
    mem = np.concatenate([u1[i1], u2[i2]], axis=0)               # [M, C]
    memlab = np.concatenate([l1[i1], l2[i2]], axis=0)            # [M]

    # --- compact the masked rows of both branches into one stream ---
    m1 = (g2 > POS_THRESH) & (g1 < g2)
    m2 = (g1 > POS_THRESH) & (g2 < g1)
    idx1 = np.nonzero(m1)[0]
    idx2 = np.nonzero(m2)[0]
    n1, n2 = len(idx1), len(idx2)
    R = n1 + n2
    NT = max(1, (((R + 127) // 128) + NCORES - 1) // NCORES)
    RT = NT * 128 * NCORES

    pos_full = (
        np.sum(f1.astype(np.float64) * f2.astype(np.float64), axis=1) / TEMP
    ).astype(np.float32)

    anchors = np.zeros((RT, C), np.float32)
    alab = np.zeros(RT, np.int64)
    posv = np.zeros(RT, np.float32)
    anchors[:n1] = f1[idx1]
    alab[:n1] = l1[idx1]
    posv[:n1] = pos_full[idx1]
    anchors[n1:R] = f2[idx2]
    alab[n1:R] = l2[idx2]
    posv[n1:R] = pos_full[idx2]

    lab_eye = np.arange(NLAB, dtype=np.int64)

    extmem = np.zeros((C, MP), np.float32)
    extmem[:, :M] = mem.T / TEMP
    extmem = extmem.astype(bf)                                   # [256, MP]

    oh_mem = np.zeros((NLAB, MP), np.float32)
    oh_mem[:, :M] = (memlab[None, :] == lab_eye[:, None])
    oh_mem[:, M:] = 1.0          # pad columns masked for every label
    eqmem = np.zeros((128, MP), np.float32)
    for i in range(NU):
        eqmem[32 * i: 32 * i + NLAB] = oh_mem
    eqmem = eqmem.astype(bf)                                     # [128, MP]

    oh_anc = -1000.0 * (alab[None, :] == lab_eye[:, None])       # [21, RT]
    eqa_full = np.zeros((128, RT), np.float32)
    for i in range(NU):
        eqa_full[32 * i: 32 * i + NLAB] = oh_anc
    eqa_full = eqa_full.astype(bf)

    exta = np.ascontiguousarray(anchors.T).astype(bf)            # [256, RT]

    RPC = NT * 128

    def pack_vec(v):    # [RPC] -> [128, NT]
        return np.ascontiguousarray(v.reshape(NT, 128).T)

    in_maps = []
    for cix in range(NCORES):
        sl = slice(cix * RPC, (cix + 1) * RPC)
        in_maps.append({
            "exta": np.ascontiguousarray(exta[:, sl]),
            "eqanc": np.ascontiguousarray(eqa_full[:, sl]),
            "extmem": extmem,
            "eqmem": eqmem,
            "npos": pack_vec(-posv[sl]),
        })
    return in_maps, NT, n1, n2


def _finalize(results, NT, n1, n2):
    RPC = NT * 128
    num1 = num2 = 0.0
    for cix, r in enumerate(results):
        ss = np.asarray(r["out"], np.float64)       # [128, NT]
        v = ss.T.reshape(RPC)                       # unpack pack_vec
        gidx = np.arange(cix * RPC, (cix + 1) * RPC)
        with np.errstate(divide="ignore", over="ignore"):
            lam = -np.log(1.0 / (v + 1.0 + EPS) + EPS)
        num1 += lam[gidx < n1].sum()
        num2 += lam[(gidx >= n1) & (gidx < n1 + n2)].sum()
    loss = num1 / (n1 + 1e-12) + num2 / (n2 + 1e-12)
    return np.float32(loss)


def _run(inputs, trace=False):
    from concourse.bass_utils import run_bass_kernel_spmd

    in_maps, NT, n1, n2 = _host_prep(inputs)
    if NT not in _cache:
        _cache[NT] = _build(NT)
    res = run_bass_kernel_spmd(
        _cache[NT], in_maps, list(range(NCORES)), trace=trace
    )
    return _finalize(res.results, NT, n1, n2), res


def kernel(**inputs):
    out, _ = _run(inputs)
    return out


def kernel_with_profile(**inputs):
    out, res = _run(inputs, trace=True)
    return out, res


# revision 30
# speedup vs baseline: 1.0671x; 1.0264x over previous
"""DirectionalContrastiveLoss on 8 TRN2 NeuronCores (Bass/Tile).

Key optimization over the dense data-parallel version: the loss only
involves anchor rows whose pos-mask is nonzero, and the masks depend
only on the host-visible pseudo_logits:
    pos_mask_1 = (g2 > 0.7) & (g1 < g2)   ~26% of rows
    pos_mask_2 = (g1 > 0.7) & (g2 < g1)   ~26% of rows (disjoint from 1)
So the host compacts the ~52% union of masked rows into one unified
anchor stream (branch-1 rows use feat1 as anchor/label1, branch-2 rows
use feat2/label2; pos = <f1,f2>/TEMP is shared and precomputed on the
host in fp64).  This cuts PE matmul, ScalarE exp, and DVE reduce work
~3.6x vs processing all 16384 rows for both branches.

Device algorithm (validated against the fp64 reference in numcheck):
- sim = anchor @ memT/TEMP - 4000*eq computed on the PE in bf16; the
  label mask rides along as one accumulating matmul per 512-col chunk
  using the full 128-partition one-hot tensors (4 stacked copies at
  32-row offsets -> bias -4000, and a 128-row stationary keeps FWL on
  so LoadStationary never bubbles the PE stream).  exp(sim-4000) == 0
  in fp32, reproducing the reference's masked exp-sum.  Memory pad
  columns (4000->4096) carry onehot=1 in every label row so they
  vanish for every anchor.
- The softmax shift is pos (not the row max): rows where exp(sim-pos)
  overflows to +inf are provably dead (sim >= pos + 88 implies the
  true logit < e^-88, so -log(sigma+EPS) = -log(EPS) either way), and
  rows that matter can never overflow.
- The device returns the raw per-row masked exp-sums SS [128, NT]; the
  host computes -log(1/(SS+1+EPS)+EPS) and the two masked means in
  fp64 (cheap O(N), and it avoids an ACT table switch for Ln plus the
  whole on-device epilogue).

TRN2 clock (HAM) handling: the PE boost (1.2 -> 2.4 GHz) only engages
after sustained busy windows and drops permanently on sub-microsecond
PE gaps, so the kernel front-loads no-dependency garbage matmuls while
the DMAs stream in (plus per-unit insurance matmuls in the first two
tiles), and the steady-state pipeline is kept strictly PE-bound.
"""
from contextlib import ExitStack

import numpy as np
import ml_dtypes

TEMP = 0.1
POS_THRESH = 0.7
EPS = 1e-8
N, C, M, NLAB = 16384, 256, 4000, 21
MP = 4096                  # memory columns padded
NCORES = 8
NU = 4                     # psum units per n-tile
UNIT = MP // NU            # 1024 (= 2 PSUM banks, J=512 chunks)

_cache = {}


def _build(NT):
    import concourse.bacc as bacc
    import concourse.tile as tile
    from concourse import mybir

    f32 = mybir.dt.float32
    bf16 = mybir.dt.bfloat16
    Act = mybir.ActivationFunctionType
    X = mybir.AxisListType.X

    RPC = NT * 128             # compacted rows per core

    # Bacc (not raw Bass): its finalize() runs generate_event_semaphores(),
    # which splits multi-sem waits into EVSEM chains — walrus allows at most
    # one sync-wait per instruction.
    nc = bacc.Bacc(None)

    exta_d = nc.declare_dram_parameter("exta", [C, RPC], bf16, isOutput=False)
    eqa_d = nc.declare_dram_parameter("eqanc", [128, RPC], bf16, isOutput=False)
    mem_d = nc.declare_dram_parameter("extmem", [C, MP], bf16, isOutput=False)
    eqm_d = nc.declare_dram_parameter("eqmem", [128, MP], bf16, isOutput=False)
    npos_d = nc.declare_dram_parameter("npos", [128, NT], f32, isOutput=False)
    out_d = nc.declare_dram_parameter("out", [128, NT], f32, isOutput=True)

    with tile.TileContext(nc) as tc, ExitStack() as ctx:
        consts = ctx.enter_context(tc.tile_pool(name="consts", bufs=1))
        small = ctx.enter_context(tc.tile_pool(name="small", bufs=3))
        psum = ctx.enter_context(
            tc.tile_pool(name="psum", bufs=NU, space="PSUM")
        )

        # ---- resident inputs ----
        # Order matters: NPOS (gates the first ScalarE exp) and tile-0's
        # matmul operands load first in small dedicated tiles, then the
        # bulk tensors.
        NPOS = consts.tile([128, NT], f32, tag="NPOS", name="NPOS")
        nc.sync.dma_start(out=NPOS[:], in_=npos_d[:])

        ea0_k, eqa0 = [], None
        for i in range(2):
            k0, k1 = i * 128, (i + 1) * 128
            t0t = consts.tile([128, 128], bf16, tag=f"ea0_{i}", name=f"ea0_{i}")
            nc.sync.dma_start(out=t0t[:], in_=exta_d[k0:k1, 0:128])
            ea0_k.append(t0t)
        eqa0 = consts.tile([128, 128], bf16, tag="eqa0", name="eqa0")
        nc.sync.dma_start(out=eqa0[:], in_=eqa_d[:, 0:128])

        memc = [[None] * NU for _ in range(2)]
        eqmc = [None] * NU
        for u in range(NU):
            c0, c1 = u * UNIT, (u + 1) * UNIT
            for i in range(2):
                k0, k1 = i * 128, (i + 1) * 128
                mt = consts.tile([128, UNIT], bf16, tag=f"mem{i}u{u}",
                                 name=f"mem{i}u{u}")
                nc.sync.dma_start(out=mt[:], in_=mem_d[k0:k1, c0:c1])
                memc[i][u] = mt
            et = consts.tile([128, UNIT], bf16, tag=f"eqmu{u}", name=f"eqmu{u}")
            nc.sync.dma_start(out=et[:], in_=eqm_d[:, c0:c1])
            eqmc[u] = et

        ea_k = []
        for i in range(2):
            k0, k1 = i * 128, (i + 1) * 128
            t1 = consts.tile([128, RPC], bf16, tag=f"ea_{i}", name=f"ea_{i}")
            nc.sync.dma_start(out=t1[:], in_=exta_d[k0:k1, :])
            ea_k.append(t1)
        eqa = consts.tile([128, RPC], bf16, tag="eqa", name="eqa")
        nc.sync.dma_start(out=eqa[:], in_=eqa_d[:])

        # ---- HAM warm-up ballast ----
        # The PE only un-throttles from 1.2 to 2.4 GHz when it sees
        # sustained busy windows, and sub-us gaps re-throttle it (often
        # permanently for a light kernel).  Garbage matmuls with no data
        # deps keep the PE lit while the real inputs stream in.
        dmm = consts.tile([128, 1024], bf16, tag="dmm", name="dmm")
        nc.vector.memset(dmm[:], 0.0)
        dvedum = consts.tile([128, 2048], f32, tag="dvedum", name="dvedum")
        nc.vector.memset(dvedum[:], 1.0)
        dscr = consts.tile([128, 1], f32, tag="dscr", name="dscr")
        pdum = psum.tile([128, UNIT], f32, tag="pu", name="pdum")
        for i in range(16):
            j = i % 2
            nc.tensor.matmul(
                pdum[:, j * 512: (j + 1) * 512],
                dmm[:, 0:128],
                dmm[:, 0:512],
                start=True,
                stop=True,
            )
        for i in range(6):
            nc.vector.reduce_sum(out=dscr[:, 0:1], in_=dvedum[:], axis=X)

        SS = consts.tile([128, NT], f32, tag="SS", name="SS")
        for t in range(NT):
            tc0, tc1 = t * 128, (t + 1) * 128
            pu = [
                psum.tile([128, UNIT], f32, tag="pu", name=f"pu{t}_{u}")
                for u in range(NU)
            ]
            S = small.tile([128, NU], f32, tag="S", name=f"S{t}")
            ea_u = ea0_k if t == 0 else ea_k
            eqa_u = eqa0 if t == 0 else eqa
            ec0, ec1 = (0, 128) if t == 0 else (tc0, tc1)
            # Per-unit: dense K=256 bf16 (2 k-tiles), the -4000*eq one-hot
            # mask matmul (full 128 rows -> FWL stays on), then exp on
            # ScalarE (in place) and row-sum on VectorE while the PE moves
            # on to the next unit.
            for u in range(NU):
                if t < 2:
                    # handoff insurance: a no-dep garbage matmul keeps the
                    # PE stream gapless if this unit's inputs are still in
                    # flight (the real matmul below overwrites the region).
                    nc.tensor.matmul(
                        pu[u][:, 0:512],
                        dmm[:, 0:128],
                        dmm[:, 0:512],
                        start=True,
                        stop=True,
                    )
                for kt in range(2):
                    lhsT = ea_u[kt][:, ec0:ec1]
                    for j in range(2):
                        nc.tensor.matmul(
                            pu[u][:, j * 512: (j + 1) * 512],
                            lhsT,
                            memc[kt][u][:, j * 512: (j + 1) * 512],
                            start=(kt == 0),
                            stop=False,
                        )
                for j in range(2):
                    nc.tensor.matmul(
                        pu[u][:, j * 512: (j + 1) * 512],
                        eqa_u[:, ec0:ec1],
                        eqmc[u][:, j * 512: (j + 1) * 512],
                        start=False,
                        stop=True,
                    )
                nc.scalar.activation(
                    out=pu[u][:],
                    in_=pu[u][:],
                    func=Act.Exp,
                    bias=NPOS[:, t: t + 1],
                    scale=1.0,
                )
            for u in range(NU):
                nc.vector.reduce_sum(out=S[:, u: u + 1], in_=pu[u][:], axis=X)
            nc.vector.reduce_sum(out=SS[:, t: t + 1], in_=S[:], axis=X)

        nc.sync.dma_start(out=out_d[:], in_=SS[:])

    nc.finalize()
    return nc


def _host_prep(inputs):
    bf = ml_dtypes.bfloat16
    f1 = np.ascontiguousarray(np.asarray(inputs["output_feat1"], np.float32))
    f2 = np.ascontiguousarray(np.asarray(inputs["output_feat2"], np.float32))
    l1 = np.asarray(inputs["pseudo_label1"], np.int64)
    l2 = np.asarray(inputs["pseudo_label2"], np.int64)
    g1 = np.asarray(inputs["pseudo_logits1"], np.float32)
    g2 = np.asarray(inputs["pseudo_logits2"], np.float32)
    ul1 = np.asarray(inputs["output_ul1"], np.float32)
    ul2 = np.asarray(inputs["output_ul2"], np.float32)
    i1 = np.asarray(inputs["selected_idx1"], np.int64)
    i2 = np.asarray(inputs["selected_idx2"], np.int64)

    b, c, h, w = ul1.shape
    u1 = ul1.transpose(0, 2, 3, 1).reshape(b * h * w, c)
    u2 = ul2.transpose(0, 2, 3, 1).reshape(b * h * w, c)
    mem = np.concatenate([u1[i1], u2[i2]], axis=0)               # [M, C]
    memlab = np.concatenate([l1[i1], l2[i2]], axis=0)            # [M]

    # --- compact the masked rows of both branches into one stream ---
    m1 = (g2 > POS_THRESH) & (g1 < g2)
    m2 = (g1 > POS_THRESH) & (g2 < g1)
    idx1 = np.nonzero(m1)[0]
    idx2 = np.nonzero(m2)[0]
    n1, n2 = len(idx1), len(idx2)
    R = n1 + n2
    NT = max(1, (((R + 127) // 128) + NCORES - 1) // NCORES)
    RT = NT * 128 * NCORES

    pos_full = (
        np.sum(f1.astype(np.float64) * f2.astype(np.float64), axis=1) / TEMP
    ).astype(np.float32)

    anchors = np.zeros((RT, C), np.float32)
    alab = np.zeros(RT, np.int64)
    posv = np.zeros(RT, np.float32)
    anchors[:n1] = f1[idx1]
    alab[:n1] = l1[idx1]
    posv[:n1] = pos_full[idx1]
    anchors[n1:R] = f2[idx2]
    alab[n1:R] = l2[idx2]
    posv[n1:R] = pos_full[idx2]

    lab_eye = np.arange(NLAB, dtype=np.int64)

    extmem = np.zeros((C, MP), np.float32)
    extmem[:, :M] = mem.T / TEMP
    extmem = extmem.astype(bf)                                   # [256, MP]

    oh_mem = np.zeros((NLAB, MP), np.float32)
    oh_mem[:, :M] = (memlab[None, :] == lab_eye[:, None])
    oh_mem[:, M:] = 1.0          # pad columns masked for every label
    eqmem = np.zeros((128, MP), np.float32)
    for i in range(NU):
        eqmem[32 * i: 32 * i + NLAB] = oh_mem
    eqmem = eqmem.astype(bf)                                     # [128, MP]

    oh_anc = -1000.0 * (alab[None, :] == lab_eye[:, None])       # [21, RT]
    eqa_full = np.zeros((128, RT), np.float32)
    for i in range(NU):
        eqa_full[32 * i: 32 * i + NLAB] = oh_anc
    eqa_full = eqa_full.astype(bf)

    exta = np.ascontiguousarray(anchors.T).astype(bf)            # [256, RT]

    RPC = NT * 128

    def pack_vec(v):    # [RPC] -> [128, NT]
        return np.ascontiguousarray(v.reshape(NT, 128).T)

    in_maps = []
    for cix in range(NCORES):
        sl = slice(cix * RPC, (cix + 1) * RPC)
        in_maps.append({
            "exta": np.ascontiguousarray(exta[:, sl]),
            "eqanc": np.ascontiguousarray(eqa_full[:, sl]),
            "extmem": extmem,
            "eqmem": eqmem,
            "npos": pack_vec(-posv[sl]),
        })
    return in_maps, NT, n1, n2


def _finalize(results, NT, n1, n2):
    RPC = NT * 128
    num1 = num2 = 0.0
    for cix, r in enumerate(results):
        ss = np.asarray(r["out"], np.float64)       # [128, NT]
        v = ss.T.reshape(RPC)                       # unpack pack_vec
        gidx = np.arange(cix * RPC, (cix + 1) * RPC)
        with np.errstate(divide="ignore", over="ignore"):
            lam = -np.log(1.0 / (v + 1.0 + EPS) + EPS)
        num1 += lam[gidx < n1].sum()
        num2 += lam[(gidx >= n1) & (gidx < n1 + n2)].sum()
    loss = num1 / (n1 + 1e-12) + num2 / (n2 + 1e-12)
    return np.float32(loss)


def _run(inputs, trace=False):
    from concourse.bass_utils import run_bass_kernel_spmd

    in_maps, NT, n1, n2 = _host_prep(inputs)
    if NT not in _cache:
        _cache[NT] = _build(NT)
    res = run_bass_kernel_spmd(
        _cache[NT], in_maps, list(range(NCORES)), trace=trace
    )
    return _finalize(res.results, NT, n1, n2), res


def kernel(**inputs):
    out, _ = _run(inputs)
    return out


def kernel_with_profile(**inputs):
    out, res = _run(inputs, trace=True)
    return out, res


# revision 34
# speedup vs baseline: 1.1080x; 1.0383x over previous
"""DirectionalContrastiveLoss on 8 TRN2 NeuronCores (Bass/Tile).

Key optimization over the dense data-parallel version: the loss only
involves anchor rows whose pos-mask is nonzero, and the masks depend
only on the host-visible pseudo_logits:
    pos_mask_1 = (g2 > 0.7) & (g1 < g2)   ~26% of rows
    pos_mask_2 = (g1 > 0.7) & (g2 < g1)   ~26% of rows (disjoint from 1)
So the host compacts the ~52% union of masked rows into one unified
anchor stream (branch-1 rows use feat1 as anchor/label1, branch-2 rows
use feat2/label2; pos = <f1,f2>/TEMP is shared and precomputed on the
host in fp64).  This cuts PE matmul, ScalarE exp, and DVE reduce work
~3.6x vs processing all 16384 rows for both branches.

Device algorithm (validated against the fp64 reference in numcheck):
- sim = anchor @ memT/TEMP - 4000*eq computed on the PE in bf16; the
  label mask rides along as one accumulating matmul per 512-col chunk
  using the full 128-partition one-hot tensors (4 stacked copies at
  32-row offsets -> bias -4000, and a 128-row stationary keeps FWL on
  so LoadStationary never bubbles the PE stream).  exp(sim-4000) == 0
  in fp32, reproducing the reference's masked exp-sum.  Memory pad
  columns (4000->4096) carry onehot=1 in every label row so they
  vanish for every anchor.
- The softmax shift is pos (not the row max): rows where exp(sim-pos)
  overflows to +inf are provably dead (sim >= pos + 88 implies the
  true logit < e^-88, so -log(sigma+EPS) = -log(EPS) either way), and
  rows that matter can never overflow.
- The device returns the raw per-row masked exp-sums SS [128, NT]; the
  host computes -log(1/(SS+1+EPS)+EPS) and the two masked means in
  fp64 (cheap O(N), and it avoids an ACT table switch for Ln plus the
  whole on-device epilogue).

TRN2 clock (HAM) handling: the PE boost (1.2 -> 2.4 GHz) only engages
after sustained busy windows and drops permanently on sub-microsecond
PE gaps, so the kernel front-loads no-dependency garbage matmuls while
the DMAs stream in (plus per-unit insurance matmuls in the first two
tiles), and the steady-state pipeline is kept strictly PE-bound.
"""
from contextlib import ExitStack

import numpy as np
import ml_dtypes

TEMP = 0.1
POS_THRESH = 0.7
EPS = 1e-8
N, C, M, NLAB = 16384, 256, 4000, 21
MP = 4096                  # memory columns padded
NCORES = 8
NU = 4                     # psum units per n-tile
UNIT = MP // NU            # 1024 (= 2 PSUM banks, J=512 chunks)

_cache = {}


def _build(NT, masksets):
    import concourse.bacc as bacc
    import concourse.tile as tile
    from concourse import mybir

    f32 = mybir.dt.float32
    bf16 = mybir.dt.bfloat16
    Act = mybir.ActivationFunctionType
    X = mybir.AxisListType.X

    RPC = NT * 128             # compacted rows per core

    # Bacc (not raw Bass): its finalize() runs generate_event_semaphores(),
    # which splits multi-sem waits into EVSEM chains — walrus allows at most
    # one sync-wait per instruction.
    nc = bacc.Bacc(None)

    exta_d = nc.declare_dram_parameter("exta", [C, RPC], bf16, isOutput=False)
    eqa_d = nc.declare_dram_parameter("eqanc", [128, RPC], bf16, isOutput=False)
    mem_d = nc.declare_dram_parameter("extmem", [C, MP], bf16, isOutput=False)
    eqm_d = nc.declare_dram_parameter("eqmem", [128, MP], bf16, isOutput=False)
    npos_d = nc.declare_dram_parameter("npos", [128, NT], f32, isOutput=False)
    out_d = nc.declare_dram_parameter("out", [128, NT], f32, isOutput=True)

    with tile.TileContext(nc) as tc, ExitStack() as ctx:
        consts = ctx.enter_context(tc.tile_pool(name="consts", bufs=1))
        small = ctx.enter_context(tc.tile_pool(name="small", bufs=3))
        psum = ctx.enter_context(
            tc.tile_pool(name="psum", bufs=NU, space="PSUM")
        )

        # ---- resident inputs ----
        # Order matters: NPOS (gates the first ScalarE exp) and tile-0's
        # matmul operands load first in small dedicated tiles, then the
        # bulk tensors.
        NPOS = consts.tile([128, NT], f32, tag="NPOS", name="NPOS")
        nc.sync.dma_start(out=NPOS[:], in_=npos_d[:])

        ea0_k, eqa0 = [], None
        for i in range(2):
            k0, k1 = i * 128, (i + 1) * 128
            t0t = consts.tile([128, 128], bf16, tag=f"ea0_{i}", name=f"ea0_{i}")
            nc.sync.dma_start(out=t0t[:], in_=exta_d[k0:k1, 0:128])
            ea0_k.append(t0t)
        eqa0 = consts.tile([128, 128], bf16, tag="eqa0", name="eqa0")
        nc.sync.dma_start(out=eqa0[:], in_=eqa_d[:, 0:128])

        memc = [[None] * NU for _ in range(2)]
        eqmc = [None] * NU
        for u in range(NU):
            c0, c1 = u * UNIT, (u + 1) * UNIT
            for i in range(2):
                k0, k1 = i * 128, (i + 1) * 128
                mt = consts.tile([128, UNIT], bf16, tag=f"mem{i}u{u}",
                                 name=f"mem{i}u{u}")
                nc.sync.dma_start(out=mt[:], in_=mem_d[k0:k1, c0:c1])
                memc[i][u] = mt
            et = consts.tile([128, UNIT], bf16, tag=f"eqmu{u}", name=f"eqmu{u}")
            nc.sync.dma_start(out=et[:], in_=eqm_d[:, c0:c1])
            eqmc[u] = et

        ea_k = []
        for i in range(2):
            k0, k1 = i * 128, (i + 1) * 128
            t1 = consts.tile([128, RPC], bf16, tag=f"ea_{i}", name=f"ea_{i}")
            nc.sync.dma_start(out=t1[:], in_=exta_d[k0:k1, :])
            ea_k.append(t1)
        eqa = consts.tile([128, RPC], bf16, tag="eqa", name="eqa")
        nc.sync.dma_start(out=eqa[:], in_=eqa_d[:])

        # ---- HAM warm-up ballast ----
        # The PE only un-throttles from 1.2 to 2.4 GHz when it sees
        # sustained busy windows, and sub-us gaps re-throttle it (often
        # permanently for a light kernel).  Garbage matmuls with no data
        # deps keep the PE lit while the real inputs stream in.
        dmm = consts.tile([128, 1024], bf16, tag="dmm", name="dmm")
        nc.vector.memset(dmm[:], 0.0)
        dvedum = consts.tile([128, 2048], f32, tag="dvedum", name="dvedum")
        nc.vector.memset(dvedum[:], 1.0)
        dscr = consts.tile([128, 1], f32, tag="dscr", name="dscr")
        pdum = psum.tile([128, UNIT], f32, tag="pu", name="pdum")
        for i in range(16):
            j = i % 2
            nc.tensor.matmul(
                pdum[:, j * 512: (j + 1) * 512],
                dmm[:, 0:128],
                dmm[:, 0:512],
                start=True,
                stop=True,
            )
        for i in range(6):
            nc.vector.reduce_sum(out=dscr[:, 0:1], in_=dvedum[:], axis=X)

        SS = consts.tile([128, NT], f32, tag="SS", name="SS")
        for t in range(NT):
            tc0, tc1 = t * 128, (t + 1) * 128
            pu = [
                psum.tile([128, UNIT], f32, tag="pu", name=f"pu{t}_{u}")
                for u in range(NU)
            ]
            S = small.tile([128, NU], f32, tag="S", name=f"S{t}")
            ea_u = ea0_k if t == 0 else ea_k
            eqa_u = eqa0 if t == 0 else eqa
            ec0, ec1 = (0, 128) if t == 0 else (tc0, tc1)
            # Per-unit: dense K=256 bf16 (2 k-tiles), then -4000*eq one-hot
            # mask matmuls only over the 512-col chunks that can contain
            # this tile's labels (anchors are dealt label-sorted round-robin
            # across cores, memory is label-sorted, so the masked columns
            # form one narrow compile-time range; full 128-row stationaries
            # keep FWL on), then exp on ScalarE (in place) and row-sum on
            # VectorE while the PE moves on to the next unit.
            mset = masksets[t]
            for u in range(NU):
                if t < 2:
                    # handoff insurance: a no-dep garbage matmul keeps the
                    # PE stream gapless if this unit's inputs are still in
                    # flight (the real matmul below overwrites the region).
                    nc.tensor.matmul(
                        pu[u][:, 0:512],
                        dmm[:, 0:128],
                        dmm[:, 0:512],
                        start=True,
                        stop=True,
                    )
                for kt in range(2):
                    lhsT = ea_u[kt][:, ec0:ec1]
                    for j in range(2):
                        nc.tensor.matmul(
                            pu[u][:, j * 512: (j + 1) * 512],
                            lhsT,
                            memc[kt][u][:, j * 512: (j + 1) * 512],
                            start=(kt == 0),
                            stop=(kt == 1) and (u * 2 + j) not in mset,
                        )
                for j in range(2):
                    if (u * 2 + j) in mset:
                        nc.tensor.matmul(
                            pu[u][:, j * 512: (j + 1) * 512],
                            eqa_u[:, ec0:ec1],
                            eqmc[u][:, j * 512: (j + 1) * 512],
                            start=False,
                            stop=True,
                        )
                nc.scalar.activation(
                    out=pu[u][:],
                    in_=pu[u][:],
                    func=Act.Exp,
                    bias=NPOS[:, t: t + 1],
                    scale=1.0,
                )
            for u in range(NU):
                nc.vector.reduce_sum(out=S[:, u: u + 1], in_=pu[u][:], axis=X)
            nc.vector.reduce_sum(out=SS[:, t: t + 1], in_=S[:], axis=X)

        nc.sync.dma_start(out=out_d[:], in_=SS[:])

    nc.finalize()
    return nc


def _host_prep(inputs):
    bf = ml_dtypes.bfloat16
    f1 = np.ascontiguousarray(np.asarray(inputs["output_feat1"], np.float32))
    f2 = np.ascontiguousarray(np.asarray(inputs["output_feat2"], np.float32))
    l1 = np.asarray(inputs["pseudo_label1"], np.int64)
    l2 = np.asarray(inputs["pseudo_label2"], np.int64)
    g1 = np.asarray(inputs["pseudo_logits1"], np.float32)
    g2 = np.asarray(inputs["pseudo_logits2"], np.float32)
    ul1 = np.asarray(inputs["output_ul1"], np.float32)
    ul2 = np.asarray(inputs["output_ul2"], np.float32)
    i1 = np.asarray(inputs["selected_idx1"], np.int64)
    i2 = np.asarray(inputs["selected_idx2"], np.int64)

    b, c, h, w = ul1.shape
    u1 = ul1.transpose(0, 2, 3, 1).reshape(b * h * w, c)
    u2 = ul2.transpose(0, 2, 3, 1).reshape(b * h * w, c)
    mem = np.concatenate([u1[i1], u2[i2]], axis=0)               # [M, C]
    memlab = np.concatenate([l1[i1], l2[i2]], axis=0)            # [M]

    # --- compact the masked rows of both branches into one stream ---
    m1 = (g2 > POS_THRESH) & (g1 < g2)
    m2 = (g1 > POS_THRESH) & (g2 < g1)
    idx1 = np.nonzero(m1)[0]
    idx2 = np.nonzero(m2)[0]
    n1, n2 = len(idx1), len(idx2)
    R = n1 + n2
    NT = max(1, (((R + 127) // 128) + NCORES - 1) // NCORES)
    RPC = NT * 128
    RT = RPC * NCORES

    pos_full = (
        np.sum(f1.astype(np.float64) * f2.astype(np.float64), axis=1) / TEMP
    ).astype(np.float32)

    anc_r = np.concatenate([f1[idx1], f2[idx2]], axis=0)         # [R, C]
    lab_r = np.concatenate([l1[idx1], l2[idx2]])                 # [R]
    pos_r = np.concatenate([pos_full[idx1], pos_full[idx2]])     # [R]

    # Deal label-sorted rows round-robin across cores: device slot s of
    # core c holds sorted row j = s*NCORES + c, so tile t on EVERY core
    # draws its labels from sorted block [t*128*NCORES, (t+1)*128*NCORES)
    # — one narrow, core-independent label range per tile.
    sidx = np.argsort(lab_r, kind="stable")
    j = np.arange(R)
    d = (j % NCORES) * RPC + (j // NCORES)
    anchors = np.zeros((RT, C), np.float32)
    alab = np.full(RT, NLAB - 1, np.int64)       # pads carry the last label
    posv = np.zeros(RT, np.float32)
    origin = np.full(RT, -1, np.int64)           # device row -> compacted row
    anchors[d] = anc_r[sidx]
    alab[d] = lab_r[sidx]
    posv[d] = pos_r[sidx]
    origin[d] = sidx

    # Label-sorted memory bank + per-label column ranges.
    morder = np.argsort(memlab, kind="stable")
    mem_s = mem[morder]
    memlab_s = memlab[morder]
    lab_eye = np.arange(NLAB, dtype=np.int64)
    mstart = np.searchsorted(memlab_s, lab_eye, side="left")
    mend = np.searchsorted(memlab_s, lab_eye, side="right")

    # Per-tile mask chunk sets (512-col granularity, shared by all cores).
    blk_lab = lab_r[sidx]
    BLK = 128 * NCORES
    masksets = []
    for t in range(NT):
        b0, b1 = t * BLK, min((t + 1) * BLK, R)
        if b0 < R:
            lo, hi = int(blk_lab[b0]), int(blk_lab[b1 - 1])
            hi = NLAB - 1 if b1 >= R else hi     # tile also holds pad rows
        else:
            lo = hi = NLAB - 1
        cs, ce = int(mstart[lo]) // 512, (int(mend[hi]) + 511) // 512
        mset = set(range(cs, ce)) | {7}          # chunk 7 covers pad cols
        for extra in range(8):                   # pad to >=6 chunks so the
            if len(mset) >= 6:                   # PE stays above ScalarE
                break
            mset.add(extra)
        masksets.append(frozenset(mset))
    masksets = tuple(masksets)

    extmem = np.zeros((C, MP), np.float32)
    extmem[:, :M] = mem_s.T / TEMP
    extmem = extmem.astype(bf)                                   # [256, MP]

    oh_mem = np.zeros((NLAB, MP), np.float32)
    oh_mem[:, :M] = (memlab_s[None, :] == lab_eye[:, None])
    oh_mem[:, M:] = 1.0          # pad columns masked for every label
    eqmem = np.zeros((128, MP), np.float32)
    for i in range(NU):
        eqmem[32 * i: 32 * i + NLAB] = oh_mem
    eqmem = eqmem.astype(bf)                                     # [128, MP]

    oh_anc = -1000.0 * (alab[None, :] == lab_eye[:, None])       # [21, RT]
    eqa_full = np.zeros((128, RT), np.float32)
    for i in range(NU):
        eqa_full[32 * i: 32 * i + NLAB] = oh_anc
    eqa_full = eqa_full.astype(bf)

    exta = np.ascontiguousarray(anchors.T).astype(bf)            # [256, RT]

    def pack_vec(v):    # [RPC] -> [128, NT]
        return np.ascontiguousarray(v.reshape(NT, 128).T)

    in_maps = []
    for cix in range(NCORES):
        sl = slice(cix * RPC, (cix + 1) * RPC)
        in_maps.append({
            "exta": np.ascontiguousarray(exta[:, sl]),
            "eqanc": np.ascontiguousarray(eqa_full[:, sl]),
            "extmem": extmem,
            "eqmem": eqmem,
            "npos": pack_vec(-posv[sl]),
        })
    meta = (n1, n2, origin)
    return in_maps, NT, masksets, meta


def _finalize(results, NT, meta):
    n1, n2, origin = meta
    RPC = NT * 128
    num1 = num2 = 0.0
    for cix, r in enumerate(results):
        ss = np.asarray(r["out"], np.float64)       # [128, NT]
        v = ss.T.reshape(RPC)                       # unpack pack_vec
        orig = origin[cix * RPC: (cix + 1) * RPC]
        with np.errstate(divide="ignore", over="ignore"):
            lam = -np.log(1.0 / (v + 1.0 + EPS) + EPS)
        real = orig >= 0
        num1 += lam[real & (orig < n1)].sum()
        num2 += lam[real & (orig >= n1)].sum()
    loss = num1 / (n1 + 1e-12) + num2 / (n2 + 1e-12)
    return np.float32(loss)


def _run(inputs, trace=False):
    from concourse.bass_utils import run_bass_kernel_spmd

    in_maps, NT, masksets, meta = _host_prep(inputs)
    key = (NT, masksets)
    if key not in _cache:
        _cache[key] = _build(NT, masksets)
    res = run_bass_kernel_spmd(
        _cache[key], in_maps, list(range(NCORES)), trace=trace
    )
    return _finalize(res.results, NT, meta), res


def kernel(**inputs):
    out, _ = _run(inputs)
    return out


def kernel_with_profile(**inputs):
    out, res = _run(inputs, trace=True)
    return out, res


# revision 36
# speedup vs baseline: 1.1438x; 1.0323x over previous
"""DirectionalContrastiveLoss on 8 TRN2 NeuronCores (Bass/Tile).

Key optimization over the dense data-parallel version: the loss only
involves anchor rows whose pos-mask is nonzero, and the masks depend
only on the host-visible pseudo_logits:
    pos_mask_1 = (g2 > 0.7) & (g1 < g2)   ~26% of rows
    pos_mask_2 = (g1 > 0.7) & (g2 < g1)   ~26% of rows (disjoint from 1)
So the host compacts the ~52% union of masked rows into one unified
anchor stream (branch-1 rows use feat1 as anchor/label1, branch-2 rows
use feat2/label2; pos = <f1,f2>/TEMP is shared and precomputed on the
host in fp64).  This cuts PE matmul, ScalarE exp, and DVE reduce work
~3.6x vs processing all 16384 rows for both branches.

Device algorithm (validated against the fp64 reference in numcheck):
- sim = anchor @ memT/TEMP - 4000*eq computed on the PE in bf16; the
  label mask rides along as accumulating matmuls over 512-col chunks
  using the full 128-partition one-hot tensors (4 stacked copies at
  32-row offsets -> bias -4000, and a 128-row stationary keeps FWL on
  so LoadStationary never bubbles the PE stream).  exp(sim-4000) == 0
  in fp32, reproducing the reference's masked exp-sum.  The memory
  bank is label-sorted and the anchors are dealt label-sorted
  round-robin across cores, so each tile only needs mask matmuls over
  the few chunks its label range can touch (padded to 6 of 8 chunks to
  keep the PE safely above ScalarE).  Memory pad columns (4000->4096)
  carry onehot=1 in every label row and chunk 7 is always masked, so
  they vanish for every anchor.
- The softmax shift is pos (not the row max): rows where exp(sim-pos)
  overflows to +inf are provably dead (sim >= pos + 88 implies the
  true logit < e^-88, so -log(sigma+EPS) = -log(EPS) either way), and
  rows that matter can never overflow.
- The device returns the raw per-row masked exp-sums SS [128, NT]; the
  host computes -log(1/(SS+1+EPS)+EPS) and the two masked means in
  fp64 (cheap O(N), and it avoids an ACT table switch for Ln plus the
  whole on-device epilogue).

TRN2 clock (HAM) handling: the PE boost (1.2 -> 2.4 GHz) only engages
after sustained busy windows and drops permanently on sub-microsecond
PE gaps, so the kernel front-loads no-dependency garbage matmuls while
the DMAs stream in (plus per-unit insurance matmuls in the first two
tiles), and the steady-state pipeline is kept strictly PE-bound.
"""
from contextlib import ExitStack

import numpy as np
import ml_dtypes

TEMP = 0.1
POS_THRESH = 0.7
EPS = 1e-8
N, C, M, NLAB = 16384, 256, 4000, 21
MP = 4096                  # memory columns padded
NCORES = 8
NU = 4                     # psum units per n-tile
UNIT = MP // NU            # 1024 (= 2 PSUM banks, J=512 chunks)

_cache = {}


def _build(NT, masksets):
    import concourse.bacc as bacc
    import concourse.tile as tile
    from concourse import mybir

    f32 = mybir.dt.float32
    bf16 = mybir.dt.bfloat16
    Act = mybir.ActivationFunctionType
    X = mybir.AxisListType.X

    RPC = NT * 128             # compacted rows per core

    # Bacc (not raw Bass): its finalize() runs generate_event_semaphores(),
    # which splits multi-sem waits into EVSEM chains — walrus allows at most
    # one sync-wait per instruction.
    nc = bacc.Bacc(None)

    exta_d = nc.declare_dram_parameter("exta", [C, RPC], bf16, isOutput=False)
    eqa_d = nc.declare_dram_parameter("eqanc", [128, RPC], bf16, isOutput=False)
    mem_d = nc.declare_dram_parameter("extmem", [C, MP], bf16, isOutput=False)
    eqm_d = nc.declare_dram_parameter("eqmem", [128, MP], bf16, isOutput=False)
    npos_d = nc.declare_dram_parameter("npos", [128, NT], f32, isOutput=False)
    out_d = nc.declare_dram_parameter("out", [128, NT], f32, isOutput=True)

    with tile.TileContext(nc) as tc, ExitStack() as ctx:
        consts = ctx.enter_context(tc.tile_pool(name="consts", bufs=1))
        small = ctx.enter_context(tc.tile_pool(name="small", bufs=3))
        psum = ctx.enter_context(
            tc.tile_pool(name="psum", bufs=NU, space="PSUM")
        )

        # ---- resident inputs ----
        # Order matters: NPOS (gates the first ScalarE exp) and tile-0's
        # matmul operands load first in small dedicated tiles, then the
        # bulk tensors.
        NPOS = consts.tile([128, NT], f32, tag="NPOS", name="NPOS")
        nc.sync.dma_start(out=NPOS[:], in_=npos_d[:])

        ea0_k, eqa0 = [], None
        for i in range(2):
            k0, k1 = i * 128, (i + 1) * 128
            t0t = consts.tile([128, 128], bf16, tag=f"ea0_{i}", name=f"ea0_{i}")
            nc.sync.dma_start(out=t0t[:], in_=exta_d[k0:k1, 0:128])
            ea0_k.append(t0t)
        eqa0 = consts.tile([128, 128], bf16, tag="eqa0", name="eqa0")
        nc.sync.dma_start(out=eqa0[:], in_=eqa_d[:, 0:128])

        memc = [[None] * NU for _ in range(2)]
        eqmc = [None] * NU
        for u in range(NU):
            c0, c1 = u * UNIT, (u + 1) * UNIT
            for i in range(2):
                k0, k1 = i * 128, (i + 1) * 128
                mt = consts.tile([128, UNIT], bf16, tag=f"mem{i}u{u}",
                                 name=f"mem{i}u{u}")
                nc.sync.dma_start(out=mt[:], in_=mem_d[k0:k1, c0:c1])
                memc[i][u] = mt
            et = consts.tile([128, UNIT], bf16, tag=f"eqmu{u}", name=f"eqmu{u}")
            nc.sync.dma_start(out=et[:], in_=eqm_d[:, c0:c1])
            eqmc[u] = et

        ea_k = []
        for i in range(2):
            k0, k1 = i * 128, (i + 1) * 128
            t1 = consts.tile([128, RPC], bf16, tag=f"ea_{i}", name=f"ea_{i}")
            nc.sync.dma_start(out=t1[:], in_=exta_d[k0:k1, :])
            ea_k.append(t1)
        eqa = consts.tile([128, RPC], bf16, tag="eqa", name="eqa")
        nc.sync.dma_start(out=eqa[:], in_=eqa_d[:])

        # ---- HAM warm-up ballast ----
        # The PE only un-throttles from 1.2 to 2.4 GHz when it sees
        # sustained busy windows, and sub-us gaps re-throttle it (often
        # permanently for a light kernel).  Garbage matmuls with no data
        # deps keep the PE lit while the real inputs stream in.
        dmm = consts.tile([128, 1024], bf16, tag="dmm", name="dmm")
        nc.vector.memset(dmm[:], 0.0)
        dvedum = consts.tile([128, 2048], f32, tag="dvedum", name="dvedum")
        nc.vector.memset(dvedum[:], 1.0)
        dscr = consts.tile([128, 1], f32, tag="dscr", name="dscr")
        pdum = psum.tile([128, UNIT], f32, tag="pu", name="pdum")
        for i in range(16):
            j = i % 2
            nc.tensor.matmul(
                pdum[:, j * 512: (j + 1) * 512],
                dmm[:, 0:128],
                dmm[:, 0:512],
                start=True,
                stop=True,
            )
        for i in range(6):
            nc.vector.reduce_sum(out=dscr[:, 0:1], in_=dvedum[:], axis=X)

        SS = consts.tile([128, NT], f32, tag="SS", name="SS")
        for t in range(NT):
            tc0, tc1 = t * 128, (t + 1) * 128
            pu = [
                psum.tile([128, UNIT], f32, tag="pu", name=f"pu{t}_{u}")
                for u in range(NU)
            ]
            S = small.tile([128, NU], f32, tag="S", name=f"S{t}")
            ea_u = ea0_k if t == 0 else ea_k
            eqa_u = eqa0 if t == 0 else eqa
            ec0, ec1 = (0, 128) if t == 0 else (tc0, tc1)
            # Per-unit: dense K=256 bf16 (2 k-tiles), then -4000*eq one-hot
            # mask matmuls only over the 512-col chunks that can contain
            # this tile's labels (anchors are dealt label-sorted round-robin
            # across cores, memory is label-sorted, so the masked columns
            # form one narrow compile-time range; full 128-row stationaries
            # keep FWL on), then exp on ScalarE (in place) and row-sum on
            # VectorE while the PE moves on to the next unit.
            mset = masksets[t]
            for u in range(NU):
                if t < 2:
                    # handoff insurance: a no-dep garbage matmul keeps the
                    # PE stream gapless if this unit's inputs are still in
                    # flight (the real matmul below overwrites the region).
                    nc.tensor.matmul(
                        pu[u][:, 0:512],
                        dmm[:, 0:128],
                        dmm[:, 0:512],
                        start=True,
                        stop=True,
                    )
                for kt in range(2):
                    lhsT = ea_u[kt][:, ec0:ec1]
                    for j in range(2):
                        nc.tensor.matmul(
                            pu[u][:, j * 512: (j + 1) * 512],
                            lhsT,
                            memc[kt][u][:, j * 512: (j + 1) * 512],
                            start=(kt == 0),
                            stop=(kt == 1) and (u * 2 + j) not in mset,
                        )
                for j in range(2):
                    if (u * 2 + j) in mset:
                        nc.tensor.matmul(
                            pu[u][:, j * 512: (j + 1) * 512],
                            eqa_u[:, ec0:ec1],
                            eqmc[u][:, j * 512: (j + 1) * 512],
                            start=False,
                            stop=True,
                        )
                # Row-sum split: ScalarE's accum_out costs +207ns per unit
                # (ACTIVATION_READ_ACCUMULATOR) but a DVE reduce of a
                # [128,1024] PSUM unit costs ~1.22us, so giving ScalarE two
                # units (ACT 4.76us/tile) and DVE two (2.6us/tile) keeps
                # both below the PE's ~4.9us.  The last tile accumulates
                # everything on ScalarE to shorten the serial drain.
                acc = u < 2 or t == NT - 1
                nc.scalar.activation(
                    out=pu[u][:],
                    in_=pu[u][:],
                    func=Act.Exp,
                    bias=NPOS[:, t: t + 1],
                    scale=1.0,
                    accum_out=S[:, u: u + 1] if acc else None,
                )
            for u in range(NU):
                if not (u < 2 or t == NT - 1):
                    nc.vector.reduce_sum(
                        out=S[:, u: u + 1], in_=pu[u][:], axis=X
                    )
            nc.vector.reduce_sum(out=SS[:, t: t + 1], in_=S[:], axis=X)

        nc.sync.dma_start(out=out_d[:], in_=SS[:])

    nc.finalize()
    return nc


def _host_prep(inputs):
    bf = ml_dtypes.bfloat16
    f1 = np.ascontiguousarray(np.asarray(inputs["output_feat1"], np.float32))
    f2 = np.ascontiguousarray(np.asarray(inputs["output_feat2"], np.float32))
    l1 = np.asarray(inputs["pseudo_label1"], np.int64)
    l2 = np.asarray(inputs["pseudo_label2"], np.int64)
    g1 = np.asarray(inputs["pseudo_logits1"], np.float32)
    g2 = np.asarray(inputs["pseudo_logits2"], np.float32)
    ul1 = np.asarray(inputs["output_ul1"], np.float32)
    ul2 = np.asarray(inputs["output_ul2"], np.float32)
    i1 = np.asarray(inputs["selected_idx1"], np.int64)
    i2 = np.asarray(inputs["selected_idx2"], np.int64)

    b, c, h, w = ul1.shape
    u1 = ul1.transpose(0, 2, 3, 1).reshape(b * h * w, c)
    u2 = ul2.transpose(0, 2, 3, 1).reshape(b * h * w, c)
    mem = np.concatenate([u1[i1], u2[i2]], axis=0)               # [M, C]
    memlab = np.concatenate([l1[i1], l2[i2]], axis=0)            # [M]

    # --- compact the masked rows of both branches into one stream ---
    m1 = (g2 > POS_THRESH) & (g1 < g2)
    m2 = (g1 > POS_THRESH) & (g2 < g1)
    idx1 = np.nonzero(m1)[0]
    idx2 = np.nonzero(m2)[0]
    n1, n2 = len(idx1), len(idx2)
    R = n1 + n2
    NT = max(1, (((R + 127) // 128) + NCORES - 1) // NCORES)
    RPC = NT * 128
    RT = RPC * NCORES

    pos_full = (
        np.sum(f1.astype(np.float64) * f2.astype(np.float64), axis=1) / TEMP
    ).astype(np.float32)

    anc_r = np.concatenate([f1[idx1], f2[idx2]], axis=0)         # [R, C]
    lab_r = np.concatenate([l1[idx1], l2[idx2]])                 # [R]
    pos_r = np.concatenate([pos_full[idx1], pos_full[idx2]])     # [R]

    # Deal label-sorted rows round-robin across cores: device slot s of
    # core c holds sorted row j = s*NCORES + c, so tile t on EVERY core
    # draws its labels from sorted block [t*128*NCORES, (t+1)*128*NCORES)
    # — one narrow, core-independent label range per tile.
    sidx = np.argsort(lab_r, kind="stable")
    j = np.arange(R)
    d = (j % NCORES) * RPC + (j // NCORES)
    anchors = np.zeros((RT, C), np.float32)
    alab = np.full(RT, NLAB - 1, np.int64)       # pads carry the last label
    posv = np.zeros(RT, np.float32)
    origin = np.full(RT, -1, np.int64)           # device row -> compacted row
    anchors[d] = anc_r[sidx]
    alab[d] = lab_r[sidx]
    posv[d] = pos_r[sidx]
    origin[d] = sidx

    # Label-sorted memory bank + per-label column ranges.
    morder = np.argsort(memlab, kind="stable")
    mem_s = mem[morder]
    memlab_s = memlab[morder]
    lab_eye = np.arange(NLAB, dtype=np.int64)
    mstart = np.searchsorted(memlab_s, lab_eye, side="left")
    mend = np.searchsorted(memlab_s, lab_eye, side="right")

    # Per-tile mask chunk sets (512-col granularity, shared by all cores).
    blk_lab = lab_r[sidx]
    BLK = 128 * NCORES
    masksets = []
    for t in range(NT):
        b0, b1 = t * BLK, min((t + 1) * BLK, R)
        if b0 < R:
            lo, hi = int(blk_lab[b0]), int(blk_lab[b1 - 1])
            hi = NLAB - 1 if b1 >= R else hi     # tile also holds pad rows
        else:
            lo = hi = NLAB - 1
        cs, ce = int(mstart[lo]) // 512, (int(mend[hi]) + 511) // 512
        mset = set(range(cs, ce)) | {7}          # chunk 7 covers pad cols
        for extra in range(8):                   # pad to >=6 chunks so the
            if len(mset) >= 6:                   # PE stays above ScalarE
                break
            mset.add(extra)
        masksets.append(frozenset(mset))
    masksets = tuple(masksets)

    extmem = np.zeros((C, MP), np.float32)
    extmem[:, :M] = mem_s.T / TEMP
    extmem = extmem.astype(bf)                                   # [256, MP]

    oh_mem = np.zeros((NLAB, MP), np.float32)
    oh_mem[:, :M] = (memlab_s[None, :] == lab_eye[:, None])
    oh_mem[:, M:] = 1.0          # pad columns masked for every label
    eqmem = np.zeros((128, MP), np.float32)
    for i in range(NU):
        eqmem[32 * i: 32 * i + NLAB] = oh_mem
    eqmem = eqmem.astype(bf)                                     # [128, MP]

    oh_anc = -1000.0 * (alab[None, :] == lab_eye[:, None])       # [21, RT]
    eqa_full = np.zeros((128, RT), np.float32)
    for i in range(NU):
        eqa_full[32 * i: 32 * i + NLAB] = oh_anc
    eqa_full = eqa_full.astype(bf)

    exta = np.ascontiguousarray(anchors.T).astype(bf)            # [256, RT]

    def pack_vec(v):    # [RPC] -> [128, NT]
        return np.ascontiguousarray(v.reshape(NT, 128).T)

    in_maps = []
    for cix in range(NCORES):
        sl = slice(cix * RPC, (cix + 1) * RPC)
        in_maps.append({
            "exta": np.ascontiguousarray(exta[:, sl]),
            "eqanc": np.ascontiguousarray(eqa_full[:, sl]),
            "extmem": extmem,
            "eqmem": eqmem,
            "npos": pack_vec(-posv[sl]),
        })
    meta = (n1, n2, origin)
    return in_maps, NT, masksets, meta


def _finalize(results, NT, meta):
    n1, n2, origin = meta
    RPC = NT * 128
    num1 = num2 = 0.0
    for cix, r in enumerate(results):
        ss = np.asarray(r["out"], np.float64)       # [128, NT]
        v = ss.T.reshape(RPC)                       # unpack pack_vec
        orig = origin[cix * RPC: (cix + 1) * RPC]
        with np.errstate(divide="ignore", over="ignore"):
            lam = -np.log(1.0 / (v + 1.0 + EPS) + EPS)
        real = orig >= 0
        num1 += lam[real & (orig < n1)].sum()
        num2 += lam[real & (orig >= n1)].sum()
    loss = num1 / (n1 + 1e-12) + num2 / (n2 + 1e-12)
    return np.float32(loss)


def _run(inputs, trace=False):
    from concourse.bass_utils import run_bass_kernel_spmd

    in_maps, NT, masksets, meta = _host_prep(inputs)
    key = (NT, masksets)
    if key not in _cache:
        _cache[key] = _build(NT, masksets)
    res = run_bass_kernel_spmd(
        _cache[key], in_maps, list(range(NCORES)), trace=trace
    )
    return _finalize(res.results, NT, meta), res


def kernel(**inputs):
    out, _ = _run(inputs)
    return out


def kernel_with_profile(**inputs):
    out, res = _run(inputs, trace=True)
    return out, res
